# revision 36
# baseline (speedup 1.0000x reference)
"""BinaryDense kernel for Trainium2: out = sign(x) @ sign(w).

Full shapes: x [8192, 4096] f32, w [4096, 4096] f32 -> out [8192, 4096] f32.

Sharding (8 cores, (i=1, j=2, k=4) grid): w split into 2 column blocks
of 2048, the contraction split into 4 slices of 1024.  Each core
computes a full-height partial product

    part = sign(x[:, ks]) @ sign(w[ks, js])    [8192, 2048]

and the host sums the 4 k-slice partials per column block.  Partials
are sums of <= 1024 values in {-1,+1}, so the f16 output container is
exact and the host-side f32 add is bit-exact.

Why k-split: per-core PE work is grid-invariant (~244 us at the fp8
DoubleRow stream rate), so the schedule goal is overlap.  With k=4 the
fully-resident w slice is only 8 MiB (~22 us of DMA) and a PSUM
generation spans just 4 k-pairs, so accumulations complete against the
w stream almost immediately and the PE rides the x stream (1 MiB / 2.8
us per m-tile vs 3.9 us of PE work per m-tile).  The deep sxt ring
(16) lets the x DMA+sign pipeline run ~16 m-tiles ahead of the PE.
Earlier grids measured slower under identical methodology: 4x2
baseline ~326 us, 2x2x2 ~315 us, 4x2 + 2-chunk-K hybrid ~363 us.

On-device per core:
  - sign(w) via ScalarE Sign LUT (exact +-1/0) cast to fp8e4, kept fully
    SBUF-resident; sign(x^T) streamed per m-tile the same way.
  - TensorE matmul in fp8 DoubleRow mode (2 k-tiles per pass), f32 PSUM
    accumulation => results are exact integers.
  - A post-schedule IR pass drops back-to-back-identical PE Ldweights
    (Tile emits one per matmul; the n-inner loop reuses each stationary
    4x), saving PE-sequencer dispatch.  A PE-isolated probe measures the
    LDW+MM stream at ~15.2 us per 16 k-pair m-tile (~238 ns/MM), i.e.
    LDWEIGHTS hides in a clean stream -- the schedule, not PE dispatch,
    is what matters.
  - One 4-bank PSUM tile per m-tile generation with a single [128, 2048]
    VectorE evict to f16 (fat_psum), then DMA to HBM; host casts/adds.

Measured on 8 axon TRN2 cores: bit-exact vs the jax reference
(relative error 0.0).  In-loop slope timing (device-side For_i, loop_n
2 vs 202, min-floor statistics, device-resident inputs): ~257 us
(median pair 258 us) vs 326 us for the previous 4x2 baseline under the
same methodology.  Alternatives measured slower on HW:
DoubleRowSwInterleave stationaries 347 us, sxt ring 32 ~262 us, 2x2x2
with these same improvements ~267-276 us (same-process A/B).  Note the
device P0-throttles under sustained benching (PE 2.4 -> ~2.0 GHz):
cool-window pairs measure 241-251 us, throttled ~290-300 us; the
fp8-DoubleRow PE stream floor at full clock is ~244 us, so the kernel
is at its roofline.
"""

import sys

if "/opt/trn_rl_repo" not in sys.path:
    sys.path.insert(0, "/opt/trn_rl_repo")

import numpy as np

P = 128
M_FULL, K_FULL, N_FULL = 8192, 4096, 4096
GRID_I, GRID_J = 4, 2  # row blocks of x  x  col blocks of w  = 8 cores
M_SH = M_FULL // GRID_I  # 2048
N_SH = N_FULL // GRID_J  # 2048
NBANK = 512  # psum bank free dim (f32)

# (i, j, k) grids over 8 cores: i = x row blocks, j = w col blocks,
# k = contraction slices (partials summed on host; k-slice partials of
# +-1 dot products are <= K/gk <= 2048, exactly representable in the
# f16 output container, so the host-side add stays bit-exact).
SHARDS = {
    "4x2": (4, 2, 1),
    "2x2x2": (2, 2, 2),
    "1x2x4": (1, 2, 4),
}

_PROGRAM_CACHE: dict = {}


def shard_dims(shard):
    gi, gj, gk = SHARDS[shard]
    return M_FULL // gi, N_FULL // gj, K_FULL // gk


def build_program(
    k_full=K_FULL,
    m_sh=M_SH,
    n_sh=N_SH,
    mm_dtype_name="float8e4",
    double_row=True,
    mm_mode="dr",  # "dr" = DoubleRow (3D lhsT), "swi" = DoubleRowSwInterleave
    g_stream=0,  # >0: hybrid schedule -- first g_stream m-tiles split K in
                 # 2 chunks w/ f16 partial eviction so PE rides the w stream
    w_head=4,  # w k-tiles issued before the x/w DMA interleave starts
    loop_n=0,
    out_dtype_name="float16",
    split_dma_rings=False,
    wstage_bufs=3,
    xstage_bufs=3,
    x_chunks=2,
    x_pre=1,
    fat_psum=False,  # classic path: one 4-bank PSUM tile + single evict
                     # per m-tile generation (fewer DVE instrs, earlier
                     # bank release)
    sw_bufs=1,  # 2 = double-buffer the resident w slice so back-to-back
                # executions overlap the next w stream with the PE tail
                # (also keeps HAM warm across invocations)
    psum_split=1,  # with fat_psum: tiles per generation (2 = two 2-bank
                   # tiles -> earlier bank release, deeper gen overlap)
    sxt_bufs=2,  # raise alongside x_pre if hoisting more m-tiles
    k_outer_pre=0,  # interleave this many hoisted m-tiles k-outer during
                    # the w stream (PSUM-capped at 2 with 4 banks each)
):
    """Build the per-core Bass program (same SPMD program on all 8 cores).

    loop_n > 0 wraps the whole body in a device-side For_i loop executing it
    loop_n times (benchmark mode: amortizes host dispatch overhead).
    """
    import contextlib

    import concourse.bass as bass  # noqa: F401
    import concourse.mybir as mybir
    import concourse.tile as tile
    from concourse import bacc

    f32 = mybir.dt.float32
    mmdt = getattr(mybir.dt, mm_dtype_name)
    # Output values are exact integers |v| <= 2048 here (sums of +-1 with
    # K=4096 random signs peak ~360), so float16 is a lossless container
    # that halves the output DMA bytes.  Host casts back to f32.
    odt = getattr(mybir.dt, out_dtype_name)
    if double_row:
        assert mm_dtype_name in ("float8e4", "float8e5")

    kt_n = k_full // P  # k tiles (32)
    mt_n = m_sh // P  # m tiles (16)
    nb_n = n_sh // NBANK  # psum banks per m tile (4)
    assert mm_mode in ("dr", "swi")
    swi = mm_mode == "swi"
    pm = (
        mybir.MatmulPerfMode.DoubleRowSwInterleave
        if swi
        else mybir.MatmulPerfMode.DoubleRow
    )

    nc = bacc.Bacc(
        "TRN2",
        target_bir_lowering=False,
        debug=False,
        num_devices=8,
    )

    # xt is packed on host: [mt, p, ko*P + m] with p = k % 128, ko = k // 128
    xt = nc.dram_tensor(
        "xt", [mt_n, P, kt_n * P], f32, kind="ExternalInput"
    ).ap()
    w = nc.dram_tensor("w", [k_full, n_sh], f32, kind="ExternalInput").ap()
    out = nc.dram_tensor("out", [m_sh, n_sh], odt, kind="ExternalOutput").ap()

    w_t = w.rearrange("(ko p) n -> p ko n", p=P)  # [128, kt_n, n_sh]
    out_t = out.rearrange("(mo p) n -> p mo n", p=P)  # [128, mt_n, n_sh]

    X_CH = kt_n // x_chunks  # k-tiles per x staging chunk

    sxt_pool_bufs = mt_n if g_stream else sxt_bufs
    with tile.TileContext(nc) as tc:
        with (
            tc.tile_pool(name="swpool", bufs=sw_bufs) as swpool,
            tc.tile_pool(name="wstage", bufs=wstage_bufs) as wstage,
            tc.tile_pool(name="xstage", bufs=max(xstage_bufs, 2 + x_pre)) as xstage,
            tc.tile_pool(name="sxtpool", bufs=sxt_pool_bufs) as sxtpool,
            tc.tile_pool(name="outpool", bufs=2) as outpool,
            tc.tile_pool(name="accpool", bufs=max(g_stream, 1)) as accpool,
            tc.tile_pool(
                name="psum",
                bufs=(2 * psum_split if fat_psum else 8),
                space="PSUM",
            ) as psump,
            tc.For_i(0, loop_n, 1) if loop_n else contextlib.nullcontext(),
        ):
            # ---- prefetch + sign the first x_pre m-tiles' x before the
            # w stream (FIFO DMA ring): the PE can then process m-tiles
            # 0..x_pre-1 while sw k-tiles arrive, instead of idling ----
            # sxt free-dim layout per k-pair kt2 (content arranged by host
            # pack_xt): dr = [kt, m] per k-tile; swi = the SwInterleave
            # stationary order [2*(127-m) + kt] so LDWEIGHTS reads pairs
            # contiguously.
            sxt_shape = [P, kt_n // 2, 2 * P] if swi else [P, kt_n, P]

            def lhsT_of(sxt, kt2):
                if swi:
                    return sxt[:, kt2, :]
                return sxt[:, 2 * kt2 : 2 * kt2 + 2, :]

            def load_sign_x(mt):
                sxt = sxtpool.tile(sxt_shape, mmdt, tag="sxt", name=f"sxt_{mt}")
                sub = X_CH * P // sxt_shape[2]  # dim-1 tiles per chunk
                for h in range(kt_n // X_CH):
                    xst = xstage.tile([P, X_CH * P], f32, tag="xst")
                    nc.sync.dma_start(
                        xst, xt[mt, :, h * X_CH * P : (h + 1) * X_CH * P]
                    )
                    nc.scalar.sign(
                        sxt[:, h * sub : (h + 1) * sub, :],
                        xst.rearrange("p (a b) -> p a b", b=sxt_shape[2]),
                    )
                return sxt

            sw = swpool.tile([P, kt_n, n_sh], mmdt)
            w_issued = 0

            def issue_w(n):
                nonlocal w_issued
                hi = min(w_issued + n, kt_n)
                for kt in range(w_issued, hi):
                    wst = wstage.tile([P, n_sh], f32, tag="wst")
                    nc.sync.dma_start(wst, w_t[:, kt, :])
                    nc.scalar.sign(sw[:, kt, :], wst)
                w_issued = hi

            def mm_tile(sxt, ps_list, kt2_lo, kt2_hi):
                # nb-inner so 4 consecutive MMs share one stationary
                # (keeps the post-schedule Ldweights dedup effective)
                for kt2 in range(kt2_lo, kt2_hi):
                    for nb in range(nb_n):
                        nc.tensor.matmul(
                            ps_list[nb],
                            lhsT=lhsT_of(sxt, kt2),
                            rhs=sw[
                                :,
                                2 * kt2 : 2 * kt2 + 2,
                                nb * NBANK : (nb + 1) * NBANK,
                            ],
                            start=(kt2 == kt2_lo),
                            stop=(kt2 == kt2_hi - 1),
                            perf_mode=pm,
                        )

            if g_stream:
                # ---- hybrid schedule.  DMA issue order: x0, then w chunk 0
                # interleaved with x1..~G/2, then w chunk 1 with the rest of
                # the streaming m-tiles, then the remaining x.  The first
                # g_stream m-tiles split K into 2 chunks with f16 partial
                # eviction (chunk partials <= 2048 are f16-exact), so their
                # PSUM generations complete against w chunk 0 instead of
                # waiting for the full w stream -- PE rides the stream. ----
                assert double_row
                G = min(g_stream, mt_n)
                kc = kt_n // 4  # k-pairs per chunk (half of kt_n//2)
                ga = max(1, G // 2)  # x m-tiles sprinkled in chunk 0

                def interleave(ws, xs):
                    out, j = [], 0
                    for i, kt in enumerate(ws):
                        out.append(("w", kt))
                        while j < len(xs) and (j + 1) * len(ws) <= (
                            i + 1
                        ) * len(xs):
                            out.append(("x", xs[j]))
                            j += 1
                    out.extend(("x", m) for m in xs[j:])
                    return out

                plan = (
                    [("x", 0)]
                    + interleave(range(0, kt_n // 2), range(1, 1 + ga))
                    + interleave(range(kt_n // 2, kt_n), range(1 + ga, G))
                    + [("x", m) for m in range(G, mt_n)]
                )
                pre = {}
                for kind, idx in plan:
                    if kind == "x":
                        pre[idx] = load_sign_x(idx)
                    else:
                        issue_w(1)  # plan lists each w k-tile once, in order

                # Phase A: chunk-0 partials for m-tiles 0..G-1
                accs = {}
                for mt in range(G):
                    acc = accpool.tile(
                        [P, n_sh], odt, tag="acc", name=f"acc_{mt}"
                    )
                    accs[mt] = acc
                    ps = [
                        psump.tile(
                            [P, NBANK], f32, tag="ps", name=f"psA_{mt}_{nb}"
                        )
                        for nb in range(nb_n)
                    ]
                    mm_tile(pre[mt], ps, 0, kc)
                    for nb in range(nb_n):
                        nc.vector.tensor_copy(
                            acc[:, nb * NBANK : (nb + 1) * NBANK], ps[nb]
                        )
                # Phase B: chunk-1 + add partial, emit output
                for mt in range(G):
                    acc = accs[mt]
                    ps = [
                        psump.tile(
                            [P, NBANK], f32, tag="ps", name=f"psB_{mt}_{nb}"
                        )
                        for nb in range(nb_n)
                    ]
                    mm_tile(pre[mt], ps, kc, 2 * kc)
                    for nb in range(nb_n):
                        nc.vector.scalar_tensor_tensor(
                            acc[:, nb * NBANK : (nb + 1) * NBANK],
                            ps[nb],
                            1.0,
                            acc[:, nb * NBANK : (nb + 1) * NBANK],
                            op0=mybir.AluOpType.mult,
                            op1=mybir.AluOpType.add,
                        )
                    nc.sync.dma_start(out_t[:, mt, :], acc)
                # Phase C: classic full-K m-tiles
                for mt in range(G, mt_n):
                    ps = [
                        psump.tile(
                            [P, NBANK], f32, tag="ps", name=f"ps_{mt}_{nb}"
                        )
                        for nb in range(nb_n)
                    ]
                    mm_tile(pre[mt], ps, 0, kt_n // 2)
                    outt = outpool.tile([P, n_sh], odt, tag="outt")
                    for nb in range(nb_n):
                        nc.vector.tensor_copy(
                            outt[:, nb * NBANK : (nb + 1) * NBANK], ps[nb]
                        )
                    nc.sync.dma_start(out_t[:, mt, :], outt)
                m_start = mt_n  # hybrid path handled everything

            if not g_stream:
                issue_w(w_head)  # land the first k-pairs before x prefetch
                pre = {mt: load_sign_x(mt) for mt in range(min(x_pre, mt_n))}
                issue_w(kt_n)
                m_start = 0

            if not g_stream and k_outer_pre:
                # k-outer across the first k_outer_pre m-tiles: each incoming
                # sw k-pair feeds all of them, so they jointly track the w
                # stream instead of serializing behind m-tile 0.
                assert double_row and k_outer_pre <= 2
                kp = min(k_outer_pre, mt_n)
                assert x_pre >= kp
                ps_pre = [
                    [
                        psump.tile(
                            [P, NBANK], f32, tag="ps", name=f"psp_{mt}_{nb}"
                        )
                        for nb in range(nb_n)
                    ]
                    for mt in range(kp)
                ]
                for kt2 in range(kt_n // 2):
                    for mt in range(kp):
                        for nb in range(nb_n):
                            nc.tensor.matmul(
                                ps_pre[mt][nb],
                                lhsT=lhsT_of(pre[mt], kt2),
                                rhs=sw[
                                    :,
                                    2 * kt2 : 2 * kt2 + 2,
                                    nb * NBANK : (nb + 1) * NBANK,
                                ],
                                start=(kt2 == 0),
                                stop=(kt2 == kt_n // 2 - 1),
                                perf_mode=pm,
                            )
                for mt in range(kp):
                    outt = outpool.tile([P, n_sh], odt, tag="outt")
                    for nb in range(nb_n):
                        nc.vector.tensor_copy(
                            outt[:, nb * NBANK : (nb + 1) * NBANK],
                            ps_pre[mt][nb],
                        )
                    nc.sync.dma_start(out_t[:, mt, :], outt)
                m_start = kp

            for mt in range(m_start, mt_n):
                sxt = pre[mt] if mt in pre else load_sign_x(mt)

                if fat_psum:
                    sw_n = n_sh // psum_split  # free cols per psum tile
                    pts = [
                        psump.tile(
                            [P, sw_n], f32, tag="ps", name=f"ps_{mt}_{i}"
                        )
                        for i in range(psum_split)
                    ]
                    bpt = nb_n // psum_split  # banks per tile
                    ps = [
                        pts[nb // bpt][
                            :, (nb % bpt) * NBANK : (nb % bpt + 1) * NBANK
                        ]
                        for nb in range(nb_n)
                    ]
                else:
                    ps = [
                        psump.tile(
                            [P, NBANK], f32, tag="ps", name=f"ps_{mt}_{nb}"
                        )
                        for nb in range(nb_n)
                    ]
                if double_row:
                    mm_tile(sxt, ps, 0, kt_n // 2)
                else:
                    for kt in range(kt_n):
                        for nb in range(nb_n):
                            nc.tensor.matmul(
                                ps[nb],
                                lhsT=sxt[:, kt, :],
                                rhs=sw[:, kt, nb * NBANK : (nb + 1) * NBANK],
                                start=(kt == 0),
                                stop=(kt == kt_n - 1),
                            )

                outt = outpool.tile([P, n_sh], odt, tag="outt")
                if fat_psum:
                    for i, pt in enumerate(pts):
                        nc.vector.tensor_copy(
                            outt[:, i * sw_n : (i + 1) * sw_n], pt
                        )
                else:
                    for nb in range(nb_n):
                        nc.vector.tensor_copy(
                            outt[:, nb * NBANK : (nb + 1) * NBANK], ps[nb]
                        )
                nc.sync.dma_start(out_t[:, mt, :], outt)

    _dedup_ldweights(nc)
    nc.compile()
    return nc


def _dedup_ldweights(nc):
    """Drop PE Ldweights that reload the exact stationary already resident.

    Tile's lowering emits one Ldweights per matmul; with an n-inner loop the
    same lhsT is reloaded 4x back-to-back.  Each Ldweights costs ~230 ns of
    PE sequencer dispatch (software decode), so the redundant ones saturate
    the PE.SEQ.  Only instructions with empty sync_info are dropped, and any
    other PE instruction (Drain, branch, ...) invalidates the tracked
    stationary, so semaphore semantics and pairing are preserved.
    """
    removed = 0
    for blk in nc.m.functions[0].blocks:
        il = blk.instructions
        last_key = None
        i = 0
        while i < len(il):
            inst = il[i]
            t = type(inst).__name__
            if t == "InstLdweights":
                key = (
                    str(inst.ins[0]),
                    str(inst.perf_mode),
                    str(inst.is_transpose),
                    str(inst.tile_position),
                    str(inst.tile_size),
                )
                si = inst.sync_info
                empty = si is None or (
                    not list(si.on_wait) and not list(si.on_update)
                )
                if key == last_key and empty:
                    il.pop(i)
                    removed += 1
                    continue
                last_key = key
            elif t == "InstMatmult":
                pass
            elif str(getattr(inst, "engine", "")) == "EngineType.PE":
                last_key = None
            i += 1
    return removed


MM_MODE = "dr"  # module defaults used by kernel(); set to the bench winner
SHARD_MODE = "1x2x4"
BUILD_KWARGS: dict = {
    "x_pre": 2,
    "fat_psum": True,
    "x_chunks": 1,
    "sxt_bufs": 16,
    "xstage_bufs": 6,
    "w_head": 0,  # x prefetch first, then the w stream (validated order)
}


def _get_program():
    key = f"main-{MM_MODE}-{SHARD_MODE}"
    if key not in _PROGRAM_CACHE:
        m_sh, n_sh, k_sh = shard_dims(SHARD_MODE)
        _PROGRAM_CACHE[key] = build_program(
            k_full=k_sh,
            m_sh=m_sh,
            n_sh=n_sh,
            mm_mode=MM_MODE,
            **BUILD_KWARGS,
        )
    return _PROGRAM_CACHE[key]


def pack_xt(x_block: np.ndarray, mm_mode="dr") -> np.ndarray:
    """[m_sh, k] row block -> per-m-tile stationary layout (partition = k%P).

    dr : [mt, p, ko*P + m]            (k-tile-major, m within)
    swi: [mt, p, kt2*2P + 2*(P-1-m) + kt]  (SwInterleave: A/B pair per m
         column interleaved, m columns reversed -- HW reads contiguously)
    """
    m_sh, k_full = x_block.shape
    if mm_mode == "swi":
        v = x_block.reshape(m_sh // P, P, k_full // (2 * P), 2, P)
        # [mt, m, kt2, kt, p] -> [mt, p, kt2, m, kt]
        v = v.transpose(0, 4, 2, 1, 3)[:, :, :, ::-1, :]
        return np.ascontiguousarray(v).reshape(m_sh // P, P, k_full)
    # target[mt, p, ko, m] = x_block[mt*P + m, ko*P + p]
    v = x_block.reshape(m_sh // P, P, k_full // P, P)  # [mt, m, ko, p]
    v = v.transpose(0, 3, 2, 1)  # [mt, p, ko, m]
    return np.ascontiguousarray(v).reshape(m_sh // P, P, k_full)


def make_in_maps(x: np.ndarray, w: np.ndarray, mm_mode="dr", shard="4x2"):
    """Shard full inputs into per-core in_maps over the (i, j, k) grid."""
    x = np.asarray(x, dtype=np.float32)
    w = np.asarray(w, dtype=np.float32)
    gi, gj, gk = SHARDS[shard]
    m_sh, n_sh, k_sh = shard_dims(shard)
    xt_shards = {}
    w_shards = {}
    in_maps = []
    for c in range(8):
        i, rem = divmod(c, gj * gk)
        j, k = divmod(rem, gk)
        if (i, k) not in xt_shards:
            xt_shards[(i, k)] = pack_xt(
                x[i * m_sh : (i + 1) * m_sh, k * k_sh : (k + 1) * k_sh],
                mm_mode,
            )
        if (k, j) not in w_shards:
            w_shards[(k, j)] = np.ascontiguousarray(
                w[k * k_sh : (k + 1) * k_sh, j * n_sh : (j + 1) * n_sh]
            )
        in_maps.append({"xt": xt_shards[(i, k)], "w": w_shards[(k, j)]})
    return in_maps


def assemble(results, shard="4x2"):
    """Sum/concat per-core blocks into the full [8192, 4096] output."""
    gi, gj, gk = SHARDS[shard]
    m_sh, n_sh, _ = shard_dims(shard)
    out = np.zeros((M_FULL, N_FULL), dtype=np.float32)
    for c in range(8):
        i, rem = divmod(c, gj * gk)
        j, k = divmod(rem, gk)
        out[i * m_sh : (i + 1) * m_sh, j * n_sh : (j + 1) * n_sh] += results[
            c
        ]["out"].astype(np.float32)
    return out


def run_on_device(x, w, trace=False, **kwargs):
    from concourse.bass_utils import run_bass_kernel_spmd

    nc = _get_program()
    in_maps = make_in_maps(x, w, MM_MODE, SHARD_MODE)
    res = run_bass_kernel_spmd(
        nc, in_maps, core_ids=list(range(8)), trace=trace, **kwargs
    )
    return res


def kernel(x: np.ndarray, w: np.ndarray) -> np.ndarray:
    res = run_on_device(x, w)
    return assemble(res.results, SHARD_MODE)

